# revision 23
# baseline (speedup 1.0000x reference)
"""HODLR matvec kernel for 8 TRN2 NeuronCores (Bass/Tile).

Sharding: node axis split into 8 contiguous slices of 32768 nodes.
Per core:
  projection  t[l,r,b] = sum_c u[l,c,r] * x[b,c]   (per block, all 8 levels)
              done in two passes: level-pairs (0,1) first, then (2,3),
              so the cross-core collective overlaps the second pass
  tree        combine L7-block partials up to coarser blocks
  A2A         exchange levels 0-2 sibling coefficients across cores
              (sender-side 0/1 masks make the combination core-invariant)
  expansion   corr[b,n] = sum_{l,r} u[l,n,r] * t_sib[l,r,b]
u/x are fed as fp8e4m3 (u scaled by USCALE; host divides the returned
correction by USCALE^2). The expansion runs fp8 DoubleRow matmuls that
contract two level-pairs (K=256) per instruction. Host computes diag*x
in fp32 and adds the device-computed correction.
"""

import os
import sys

sys.path.insert(0, "/opt/trn_rl_repo")

import numpy as np
import ml_dtypes

BF16 = ml_dtypes.bfloat16
FP8 = ml_dtypes.float8_e4m3

B = 64
N = 262144
NCORES = 8
M = N // NCORES          # 32768 nodes per core
R = 64
DEPTH = 8
CH = M // 128            # 256 chunks of 128 nodes
NB7 = M // 1024          # 32 L7 blocks (1024 nodes each)
USCALE = 64.0            # u is fed as u*USCALE in fp8 (e4m3 max finite 240)

_cached = {}


def _build_bass():
    import concourse.bacc as bacc
    import concourse.tile as tile
    import concourse.mybir as mybir
    from contextlib import ExitStack

    BF = mybir.dt.bfloat16
    F8 = mybir.dt.float8e4
    F32 = mybir.dt.float32
    ADD = mybir.AluOpType.add
    MULT = mybir.AluOpType.mult

    nc = bacc.Bacc(
        "TRN2",
        target_bir_lowering=False,
        debug=False,
        enable_asserts=False,
        num_devices=NCORES,
    )

    xt_d = nc.dram_tensor("xt", [128, CH, B], F8, kind="ExternalInput").ap()
    # u packed phase-major: [:, ph, k, :] holds level-pairs (2ph, 2ph+1)
    u_d = nc.dram_tensor("u", [128, 2, CH, 256], F8, kind="ExternalInput").ap()
    ut_d = nc.dram_tensor("ut", [4, 128, M], F8, kind="ExternalInput").ap()
    mA_d = nc.dram_tensor("maskA", [128, 8, B], BF, kind="ExternalInput").ap()
    mB_d = nc.dram_tensor("maskB", [64, 8, B], BF, kind="ExternalInput").ap()
    corr_d = nc.dram_tensor("corr", [B, M], BF, kind="ExternalOutput").ap()

    with tile.TileContext(nc) as tc, ExitStack() as ctx:
        const = ctx.enter_context(tc.tile_pool(name="const", bufs=1))
        upool = ctx.enter_context(tc.tile_pool(name="upool", bufs=4))
        pp = ctx.enter_context(tc.tile_pool(name="pp", bufs=4, space="PSUM"))
        treep = ctx.enter_context(tc.tile_pool(name="treep", bufs=1))
        statp = ctx.enter_context(tc.tile_pool(name="statp", bufs=1))
        utp = ctx.enter_context(tc.tile_pool(name="utp", bufs=3))
        ep = ctx.enter_context(tc.tile_pool(name="ep", bufs=4, space="PSUM"))
        yp = ctx.enter_context(tc.tile_pool(name="yp", bufs=2))
        dram = ctx.enter_context(tc.tile_pool(name="dram", bufs=1, space="DRAM"))

        xt = const.tile([128, CH, B], F8, tag="xt")
        for xq in range(4):
            nc.gpsimd.dma_start(
                xt[:, 64 * xq : 64 * (xq + 1), :],
                xt_d[:, 64 * xq : 64 * (xq + 1), :],
            )
        mA = const.tile([128, 8, B], BF, tag="mA")
        nc.scalar.dma_start(mA[:], mA_d[:])
        mB = const.tile([64, 8, B], BF, tag="mB")
        nc.scalar.dma_start(mB[:], mB_d[:])

        # prefetch first expansion ut tiles early (scalar ring)
        ut_pre = []
        for f in range(2):
            t_ = utp.tile([128, 2, 4096], F8, tag=f"utf{f}", name=f"ut_pre{f}")
            for j in range(2):
                nc.scalar.dma_start(t_[:, j, :], ut_d[2 * f + j, :, 0:4096])
            ut_pre.append(t_)

        # ---------------- projection + per-pair trees ----------------
        # G[(q, sz)][j]: [128, 64] bf16; rows 0:64 -> level 2q, rows
        # 64:128 -> level 2q+1 of the j-th block of `sz` nodes (t^T
        # layout [(l,r), b], scaled by USCALE).
        need_top = {3: 2048, 2: 8192, 1: 32768, 0: 32768}
        G = {}

        def tcopy(i, out, in_):
            (nc.vector if i % 2 == 0 else nc.any).tensor_copy(out, in_)

        def tadd(i, out, a, b):
            if i % 2 == 0:
                nc.vector.tensor_tensor(out, a, b, op=ADD)
            else:
                nc.any.tensor_add(out, a, b)

        last_u = [None]
        for ph, qs in ((0, (0, 1)), (1, (2, 3))):
            psums = {}  # (q, j) -> psum tile kept for direct tree combine
            for j in range(NB7):
                if j % 4 == 0:
                    u_t2 = upool.tile(
                        [128, 32, 256], F8, tag="u_in", name=f"u_t{ph}_{j}"
                    )
                    nc.sync.dma_start(
                        u_t2[:], u_d[:, ph, 8 * j : 8 * j + 32, :]
                    )
                u_t = u_t2[:, 8 * (j % 4) : 8 * (j % 4) + 8, :]
                if ph == 1:
                    last_u[0] = u_t2
                for qi, q in enumerate(qs):
                    ps = pp.tile([128, B], F32, tag="proj", name=f"ps{q}_{j}")
                    for ki in range(8):
                        k = 8 * j + ki
                        nc.tensor.matmul(
                            ps[:],
                            u_t[:, ki, 128 * qi : 128 * qi + 128],
                            xt[:, k, :],
                            start=(ki == 0),
                            stop=(ki == 7),
                        )
                    if q == 3:
                        # pair 3 needs the 1024-blocks themselves
                        g = treep.tile([128, B], BF, tag=f"G3_1024_{j}")
                        tcopy(j, g[:], ps[:])
                        G.setdefault((3, 1024), []).append(g)
                    else:
                        if j % 2 == 0:
                            # drain even block to SBUF (an op may read at
                            # most one PSUM input)
                            tmp = treep.tile(
                                [128, B], BF, tag=f"T{q}_{j // 2}",
                                name=f"T{q}_{j // 2}",
                            )
                            tcopy(j // 2 + q, tmp[:], ps[:])
                            psums[(q, j)] = tmp
                        else:
                            g2 = treep.tile(
                                [128, B], BF, tag=f"G{q}_2048_{j // 2}"
                            )
                            tadd(
                                j // 2,
                                g2[:],
                                psums.pop((q, j - 1))[:],
                                ps[:],
                            )
                            G.setdefault((q, 2048), []).append(g2)
            # tree up
            for q in qs:
                if q == 3:
                    lst = G[(3, 1024)]
                    nxt = []
                    for j in range(len(lst) // 2):
                        g2 = treep.tile([128, B], BF, tag=f"G3_2048_{j}")
                        tadd(j, g2[:], lst[2 * j][:], lst[2 * j + 1][:])
                        nxt.append(g2)
                    G[(3, 2048)] = nxt
                sz = 2048
                while sz < need_top[q]:
                    lst = G[(q, sz)]
                    nxt = []
                    for j in range(len(lst) // 2):
                        g2 = treep.tile([128, B], BF, tag=f"G{q}_{2 * sz}_{j}")
                        tadd(j + q, g2[:], lst[2 * j][:], lst[2 * j + 1][:])
                        nxt.append(g2)
                    G[(q, 2 * sz)] = nxt
                    sz *= 2

        # ------------- collective (deferred past the u stream) -------------
        # AllGather the raw level-0..2 partials; gate it on the tail of the
        # phase-1 u stream (via a zero-add) so its SDMA data plane doesn't
        # steal bandwidth mid-projection. Masks apply on the receive side.
        A0 = G[(0, 32768)][0]
        Bt = G[(1, 32768)][0]
        zdep = statp.tile([128, B], BF, tag="zdep")
        nc.vector.tensor_scalar_mul(zdep[:], last_u[0][:, 0, 0:B], 0.0)
        A = statp.tile([128, B], BF, tag="Agate")
        nc.vector.tensor_tensor(A[:], A0[:], zdep[:], op=ADD)
        b_in = dram.tile([192, B], BF, tag="b_in")
        b_out = dram.tile([8, 192, B], BF, tag="b_out", addr_space="Shared")
        nc.scalar.dma_start(b_in[0:128, :], A[:])
        nc.scalar.dma_start(b_in[128:192, :], Bt[0:64, :])
        nc.gpsimd.collective_compute(
            "AllGather",
            mybir.AluOpType.bypass,
            replica_groups=[list(range(NCORES))],
            ins=[b_in.opt()],
            outs=[b_out.opt()],
        )
        recvA = statp.tile([128, 8, B], BF, tag="recvA")
        recvB = statp.tile([64, 8, B], BF, tag="recvB")
        for k in range(8):
            nc.scalar.dma_start(recvA[:, k, :], b_out[k, 0:128, :])
            nc.scalar.dma_start(recvB[:, k, :], b_out[k, 128:192, :])
        mskA = statp.tile([128, 8, B], BF, tag="mskA")
        mskB = statp.tile([64, 8, B], BF, tag="mskB")
        for k in range(8):
            nc.vector.tensor_tensor(
                mskA[:, k, :], recvA[:, k, :], mA[:, k, :], op=MULT
            )
            nc.vector.tensor_tensor(
                mskB[:, k, :], recvB[:, k, :], mB[:, k, :], op=MULT
            )
        tallA = statp.tile([128, B], BF, tag="tallA")
        tallB = statp.tile([64, B], BF, tag="tallB")
        nc.vector.tensor_tensor(tallA[:], mskA[:, 0, :], mskA[:, 1, :], op=ADD)
        nc.vector.tensor_tensor(tallB[:], mskB[:, 0, :], mskB[:, 1, :], op=ADD)
        for k in range(2, 8):
            nc.vector.tensor_tensor(tallA[:], tallA[:], mskA[:, k, :], op=ADD)
            nc.vector.tensor_tensor(tallB[:], tallB[:], mskB[:, k, :], op=ADD)

        # ---------------- expansion stationaries (fp8, DoubleRow) --------
        # statf[q-pair-fuse] tiles [128, 2, B]: [:, j, :] holds the
        # stationary of level-pair (2*fuse + j); rows 0:64 = t_sib at the
        # even level of that pair, rows 64:128 = at the odd level.
        statf01 = []
        for m3 in range(2):
            s = statp.tile([128, 2, B], F8, tag=f"sf01_{m3}", name=f"sf01_{m3}")
            nc.vector.tensor_copy(s[:, 0, :], tallA[:])
            nc.vector.tensor_copy(s[0:64, 1, :], tallB[:])
            nc.vector.tensor_copy(
                s[64:128, 1, :], G[(1, 16384)][m3 ^ 1][64:128, :]
            )
            statf01.append(s)
        statf23 = []
        for m7 in range(NB7):
            s = statp.tile([128, 2, B], F8, tag=f"sf23_{m7}", name=f"sf23_{m7}")
            m5 = m7 // 4
            nc.vector.tensor_copy(
                s[0:64, 0, :], G[(2, 8192)][(m5 // 2) ^ 1][0:64, :]
            )
            nc.vector.tensor_copy(
                s[64:128, 0, :], G[(2, 4096)][m5 ^ 1][64:128, :]
            )
            nc.vector.tensor_copy(
                s[0:64, 1, :], G[(3, 2048)][(m7 // 2) ^ 1][0:64, :]
            )
            nc.vector.tensor_copy(
                s[64:128, 1, :], G[(3, 1024)][m7 ^ 1][64:128, :]
            )
            statf23.append(s)

        # ---------------- expansion (DoubleRow fp8) ----------------
        DR = mybir.MatmulPerfMode.DoubleRow
        for gb in range(8):  # 8 blocks of 8 groups x 512 nodes
            if gb == 0:
                ut_t = ut_pre
            else:
                ut_t = [
                    utp.tile(
                        [128, 2, 4096], F8, tag=f"utf{f}", name=f"utf{f}_{gb}"
                    )
                    for f in range(2)
                ]
                for f in range(2):
                    for j in range(2):
                        nc.scalar.dma_start(
                            ut_t[f][:, j, :],
                            ut_d[2 * f + j, :, 4096 * gb : 4096 * (gb + 1)],
                        )
            y_sb = yp.tile([B, 4096], BF, tag="y")
            for gg in range(8):
                g = 8 * gb + gg
                eps = ep.tile([B, 512], F32, tag="exp", name=f"eps{g}")
                sl = slice(512 * gg, 512 * (gg + 1))
                nc.tensor.matmul(
                    eps[:], statf23[g // 2][:], ut_t[1][:, :, sl],
                    start=True, stop=False, perf_mode=DR,
                )
                nc.tensor.matmul(
                    eps[:], statf01[g // 32][:], ut_t[0][:, :, sl],
                    start=False, stop=True, perf_mode=DR,
                )
                if gg % 2 == 0:
                    nc.vector.tensor_copy(y_sb[:, sl], eps[:])
                else:
                    nc.any.tensor_copy(y_sb[:, sl], eps[:])
            nc.sync.dma_start(corr_d[:, 4096 * gb : 4096 * (gb + 1)], y_sb[:])

    nc.compile()
    return nc


def _pack_inputs(x, diag, u):
    """Build per-core input maps. x (B,N,1) f32, u (DEPTH,N,R) f32."""
    in_maps = []
    x2 = np.asarray(x).reshape(B, N)
    u3 = np.asarray(u)
    for c in range(NCORES):
        base = c * M
        xsl = x2[:, base : base + M]                      # (B, M)
        usl = u3[:, base : base + M, :] * USCALE          # (8, M, 64)
        xt = np.ascontiguousarray(
            xsl.T.reshape(CH, 128, B).transpose(1, 0, 2)
        ).astype(FP8)                                     # [128, CH, B]
        up = np.ascontiguousarray(
            usl.transpose(1, 0, 2).reshape(M, 512)        # [n, l*64+r]
            .reshape(CH, 128, 2, 256)
            .transpose(1, 2, 0, 3)
        ).astype(FP8)                                     # [128, 2, CH, 256]
        utp = np.ascontiguousarray(
            usl.transpose(0, 2, 1).reshape(4, 128, M)
        ).astype(FP8)                                     # [4, 128, M]
        # masks: mask[d, l] = 1 iff this core c is in the level-l sibling
        # block of destination core d.
        mA = np.zeros((128, 8, B), dtype=BF16)
        mB = np.zeros((64, 8, B), dtype=BF16)
        for d in range(8):
            if (c // 4) == ((d // 4) ^ 1):
                mA[0:64, d, :] = 1.0   # level 0
            if (c // 2) == ((d // 2) ^ 1):
                mA[64:128, d, :] = 1.0  # level 1
            if c == d ^ 1:
                mB[:, d, :] = 1.0       # level 2
        in_maps.append({"xt": xt, "u": up, "ut": utp, "maskA": mA, "maskB": mB})
    return in_maps


last_results = None


def kernel(x, diag, u):
    global last_results
    from concourse.bass_utils import run_bass_kernel_spmd

    if "nc" not in _cached:
        _cached["nc"] = _build_bass()
    nc = _cached["nc"]

    in_maps = _pack_inputs(x, diag, u)
    res = run_bass_kernel_spmd(nc, in_maps, core_ids=list(range(NCORES)))
    last_results = res

    x2 = np.asarray(x, dtype=np.float32).reshape(B, N)
    d2 = np.asarray(diag, dtype=np.float32).reshape(1, N)
    y = d2 * x2
    inv = 1.0 / (USCALE * USCALE)
    for c in range(NCORES):
        y[:, c * M : (c + 1) * M] += res.results[c]["corr"].astype(np.float32) * inv
    return y.reshape(B, N, 1).astype(np.float32)


# revision 25
# speedup vs baseline: 1.2200x; 1.2200x over previous
"""HODLR matvec kernel for 8 TRN2 NeuronCores (Bass/Tile).

Sharding: node axis split into 8 contiguous slices of 32768 nodes.
Per core:
  projection  t[l,r,b] = sum_c u[l,c,r] * x[b,c]   (per block, all 8 levels)
              done in two passes: level-pairs (0,1) first, then (2,3),
              so the cross-core collective overlaps the second pass
  tree        combine L7-block partials up to coarser blocks
  A2A         exchange levels 0-2 sibling coefficients across cores
              (sender-side 0/1 masks make the combination core-invariant)
  expansion   corr[b,n] = sum_{l,r} u[l,n,r] * t_sib[l,r,b]
u/x are fed as fp8e4m3 (u scaled by USCALE; host divides the returned
correction by USCALE^2). The expansion runs fp8 DoubleRow matmuls that
contract two level-pairs (K=256) per instruction. Host computes diag*x
in fp32 and adds the device-computed correction.
"""

import os
import sys

sys.path.insert(0, "/opt/trn_rl_repo")

import numpy as np
import ml_dtypes

BF16 = ml_dtypes.bfloat16
FP8 = ml_dtypes.float8_e4m3

B = 64
N = 262144
NCORES = 8
M = N // NCORES          # 32768 nodes per core
R = 64
DEPTH = 8
CH = M // 128            # 256 chunks of 128 nodes
NB7 = M // 1024          # 32 L7 blocks (1024 nodes each)
USCALE = 64.0            # u is fed as u*USCALE in fp8 (e4m3 max finite 240)

_cached = {}


def _build_bass():
    import concourse.bacc as bacc
    import concourse.tile as tile
    import concourse.mybir as mybir
    from contextlib import ExitStack

    BF = mybir.dt.bfloat16
    F8 = mybir.dt.float8e4
    F32 = mybir.dt.float32
    ADD = mybir.AluOpType.add
    MULT = mybir.AluOpType.mult

    nc = bacc.Bacc(
        "TRN2",
        target_bir_lowering=False,
        debug=False,
        enable_asserts=False,
        num_devices=NCORES,
    )

    xt_d = nc.dram_tensor("xt", [128, CH, B], F8, kind="ExternalInput").ap()
    # u packed phase-major: [:, ph, k, :] holds level-pairs (2ph, 2ph+1)
    u_d = nc.dram_tensor("u", [128, 2, CH, 256], F8, kind="ExternalInput").ap()
    ut_d = nc.dram_tensor("ut", [4, 128, M], F8, kind="ExternalInput").ap()
    mA_d = nc.dram_tensor("maskA", [128, 8, B], BF, kind="ExternalInput").ap()
    mB_d = nc.dram_tensor("maskB", [64, 8, B], BF, kind="ExternalInput").ap()
    corr_d = nc.dram_tensor("corr", [B, M], BF, kind="ExternalOutput").ap()

    with tile.TileContext(nc) as tc, ExitStack() as ctx:
        const = ctx.enter_context(tc.tile_pool(name="const", bufs=1))
        upool = ctx.enter_context(tc.tile_pool(name="upool", bufs=4))
        pp = ctx.enter_context(tc.tile_pool(name="pp", bufs=4, space="PSUM"))
        treep = ctx.enter_context(tc.tile_pool(name="treep", bufs=1))
        statp = ctx.enter_context(tc.tile_pool(name="statp", bufs=1))
        utp = ctx.enter_context(tc.tile_pool(name="utp", bufs=3))
        ep = ctx.enter_context(tc.tile_pool(name="ep", bufs=4, space="PSUM"))
        yp = ctx.enter_context(tc.tile_pool(name="yp", bufs=3))
        dram = ctx.enter_context(tc.tile_pool(name="dram", bufs=1, space="DRAM"))

        xt = const.tile([128, CH, B], F8, tag="xt")
        for xq in range(4):
            nc.gpsimd.dma_start(
                xt[:, 64 * xq : 64 * (xq + 1), :],
                xt_d[:, 64 * xq : 64 * (xq + 1), :],
            )
        mA = const.tile([128, 8, B], BF, tag="mA")
        nc.scalar.dma_start(mA[:], mA_d[:])
        mB = const.tile([64, 8, B], BF, tag="mB")
        nc.scalar.dma_start(mB[:], mB_d[:])

        # prefetch first expansion ut tiles early (scalar ring)
        ut_pre = []
        for f in range(2):
            t_ = utp.tile([128, 2, 4096], F8, tag=f"utf{f}", name=f"ut_pre{f}")
            for j in range(2):
                nc.scalar.dma_start(t_[:, j, :], ut_d[2 * f + j, :, 0:4096])
            ut_pre.append(t_)

        # ---------------- projection + per-pair trees ----------------
        # G[(q, sz)][j]: [128, 64] bf16; rows 0:64 -> level 2q, rows
        # 64:128 -> level 2q+1 of the j-th block of `sz` nodes (t^T
        # layout [(l,r), b], scaled by USCALE).
        need_top = {3: 2048, 2: 8192, 1: 32768, 0: 32768}
        G = {}

        def tcopy(i, out, in_):
            (nc.vector if i % 2 == 0 else nc.any).tensor_copy(out, in_)

        def tadd(i, out, a, b):
            if i % 2 == 0:
                nc.vector.tensor_tensor(out, a, b, op=ADD)
            else:
                nc.any.tensor_add(out, a, b)

        for ph, qs in ((0, (0, 1)), (1, (2, 3))):
            psums = {}  # (q, j) -> psum tile kept for direct tree combine
            for j in range(NB7):
                if j % 4 == 0:
                    u_t2 = upool.tile(
                        [128, 32, 256], F8, tag="u_in", name=f"u_t{ph}_{j}"
                    )
                    nc.sync.dma_start(
                        u_t2[:], u_d[:, ph, 8 * j : 8 * j + 32, :]
                    )
                u_t = u_t2[:, 8 * (j % 4) : 8 * (j % 4) + 8, :]
                for qi, q in enumerate(qs):
                    ps = pp.tile([128, B], F32, tag="proj", name=f"ps{q}_{j}")
                    for ki in range(8):
                        k = 8 * j + ki
                        nc.tensor.matmul(
                            ps[:],
                            u_t[:, ki, 128 * qi : 128 * qi + 128],
                            xt[:, k, :],
                            start=(ki == 0),
                            stop=(ki == 7),
                        )
                    if q == 3:
                        # pair 3 needs the 1024-blocks themselves
                        g = treep.tile([128, B], BF, tag=f"G3_1024_{j}")
                        tcopy(j, g[:], ps[:])
                        G.setdefault((3, 1024), []).append(g)
                    else:
                        if j % 2 == 0:
                            # drain even block to SBUF (an op may read at
                            # most one PSUM input)
                            tmp = treep.tile(
                                [128, B], BF, tag=f"T{q}_{j // 2}",
                                name=f"T{q}_{j // 2}",
                            )
                            tcopy(j // 2 + q, tmp[:], ps[:])
                            psums[(q, j)] = tmp
                        else:
                            g2 = treep.tile(
                                [128, B], BF, tag=f"G{q}_2048_{j // 2}"
                            )
                            tadd(
                                j // 2,
                                g2[:],
                                psums.pop((q, j - 1))[:],
                                ps[:],
                            )
                            G.setdefault((q, 2048), []).append(g2)
            # tree up
            for q in qs:
                if q == 3:
                    lst = G[(3, 1024)]
                    nxt = []
                    for j in range(len(lst) // 2):
                        g2 = treep.tile([128, B], BF, tag=f"G3_2048_{j}")
                        tadd(j, g2[:], lst[2 * j][:], lst[2 * j + 1][:])
                        nxt.append(g2)
                    G[(3, 2048)] = nxt
                sz = 2048
                while sz < need_top[q]:
                    lst = G[(q, sz)]
                    nxt = []
                    for j in range(len(lst) // 2):
                        g2 = treep.tile([128, B], BF, tag=f"G{q}_{2 * sz}_{j}")
                        tadd(j + q, g2[:], lst[2 * j][:], lst[2 * j + 1][:])
                        nxt.append(g2)
                    G[(q, 2 * sz)] = nxt
                    sz *= 2

            if ph == 0:
                # ------------- collective (overlaps phase 1) -------------
                # AllGather the raw level-0..2 partials; apply the
                # sibling-selection masks on the receive side (the
                # sibling relation is symmetric, so the same masks work).
                A = G[(0, 32768)][0]
                Bt = G[(1, 32768)][0]
                b_in = dram.tile([192, B], BF, tag="b_in")
                b_out = dram.tile(
                    [8, 192, B], BF, tag="b_out", addr_space="Shared"
                )
                nc.scalar.dma_start(b_in[0:128, :], A[:])
                nc.scalar.dma_start(b_in[128:192, :], Bt[0:64, :])
                nc.gpsimd.collective_compute(
                    "AllGather",
                    mybir.AluOpType.bypass,
                    replica_groups=[list(range(NCORES))],
                    ins=[b_in.opt()],
                    outs=[b_out.opt()],
                )
                recvA = statp.tile([128, 8, B], BF, tag="recvA")
                recvB = statp.tile([64, 8, B], BF, tag="recvB")
                for k in range(8):
                    nc.scalar.dma_start(recvA[:, k, :], b_out[k, 0:128, :])
                    nc.scalar.dma_start(recvB[:, k, :], b_out[k, 128:192, :])
                # masked receive-combine
                mskA = statp.tile([128, 8, B], BF, tag="mskA")
                mskB = statp.tile([64, 8, B], BF, tag="mskB")
                for k in range(8):
                    nc.vector.tensor_tensor(
                        mskA[:, k, :], recvA[:, k, :], mA[:, k, :], op=MULT
                    )
                    nc.vector.tensor_tensor(
                        mskB[:, k, :], recvB[:, k, :], mB[:, k, :], op=MULT
                    )
                tallA = statp.tile([128, B], BF, tag="tallA")
                tallB = statp.tile([64, B], BF, tag="tallB")
                nc.vector.tensor_tensor(
                    tallA[:], mskA[:, 0, :], mskA[:, 1, :], op=ADD
                )
                nc.vector.tensor_tensor(
                    tallB[:], mskB[:, 0, :], mskB[:, 1, :], op=ADD
                )
                for k in range(2, 8):
                    nc.vector.tensor_tensor(
                        tallA[:], tallA[:], mskA[:, k, :], op=ADD
                    )
                    nc.vector.tensor_tensor(
                        tallB[:], tallB[:], mskB[:, k, :], op=ADD
                    )

        # ---------------- expansion stationaries (fp8, DoubleRow) --------
        # statf[q-pair-fuse] tiles [128, 2, B]: [:, j, :] holds the
        # stationary of level-pair (2*fuse + j); rows 0:64 = t_sib at the
        # even level of that pair, rows 64:128 = at the odd level.
        statf01 = []
        for m3 in range(2):
            s = statp.tile([128, 2, B], F8, tag=f"sf01_{m3}", name=f"sf01_{m3}")
            nc.vector.tensor_copy(s[:, 0, :], tallA[:])
            nc.vector.tensor_copy(s[0:64, 1, :], tallB[:])
            nc.vector.tensor_copy(
                s[64:128, 1, :], G[(1, 16384)][m3 ^ 1][64:128, :]
            )
            statf01.append(s)
        statf23 = []
        for m7 in range(NB7):
            s = statp.tile([128, 2, B], F8, tag=f"sf23_{m7}", name=f"sf23_{m7}")
            m5 = m7 // 4
            nc.vector.tensor_copy(
                s[0:64, 0, :], G[(2, 8192)][(m5 // 2) ^ 1][0:64, :]
            )
            nc.vector.tensor_copy(
                s[64:128, 0, :], G[(2, 4096)][m5 ^ 1][64:128, :]
            )
            nc.vector.tensor_copy(
                s[0:64, 1, :], G[(3, 2048)][(m7 // 2) ^ 1][0:64, :]
            )
            nc.vector.tensor_copy(
                s[64:128, 1, :], G[(3, 1024)][m7 ^ 1][64:128, :]
            )
            statf23.append(s)

        # ---------------- expansion (DoubleRow fp8) ----------------
        DR = mybir.MatmulPerfMode.DoubleRow
        for gb in range(8):  # 8 blocks of 8 groups x 512 nodes
            if gb == 0:
                ut_t = ut_pre
            else:
                ut_t = [
                    utp.tile(
                        [128, 2, 4096], F8, tag=f"utf{f}", name=f"utf{f}_{gb}"
                    )
                    for f in range(2)
                ]
                for f in range(2):
                    for j in range(2):
                        nc.scalar.dma_start(
                            ut_t[f][:, j, :],
                            ut_d[2 * f + j, :, 4096 * gb : 4096 * (gb + 1)],
                        )
            y_sb = yp.tile([B, 4096], BF, tag="y")
            for gg in range(8):
                g = 8 * gb + gg
                eps = ep.tile([B, 512], F32, tag="exp", name=f"eps{g}")
                sl = slice(512 * gg, 512 * (gg + 1))
                nc.tensor.matmul(
                    eps[:], statf23[g // 2][:], ut_t[1][:, :, sl],
                    start=True, stop=False, perf_mode=DR,
                )
                nc.tensor.matmul(
                    eps[:], statf01[g // 32][:], ut_t[0][:, :, sl],
                    start=False, stop=True, perf_mode=DR,
                )
                nc.vector.tensor_copy(y_sb[:, sl], eps[:])
            nc.sync.dma_start(corr_d[:, 4096 * gb : 4096 * (gb + 1)], y_sb[:])

    nc.compile()
    return nc


def _pack_inputs(x, diag, u):
    """Build per-core input maps. x (B,N,1) f32, u (DEPTH,N,R) f32."""
    in_maps = []
    x2 = np.asarray(x).reshape(B, N)
    u3 = np.asarray(u)
    for c in range(NCORES):
        base = c * M
        xsl = x2[:, base : base + M]                      # (B, M)
        usl = u3[:, base : base + M, :] * USCALE          # (8, M, 64)
        xt = np.ascontiguousarray(
            xsl.T.reshape(CH, 128, B).transpose(1, 0, 2)
        ).astype(FP8)                                     # [128, CH, B]
        up = np.ascontiguousarray(
            usl.transpose(1, 0, 2).reshape(M, 512)        # [n, l*64+r]
            .reshape(CH, 128, 2, 256)
            .transpose(1, 2, 0, 3)
        ).astype(FP8)                                     # [128, 2, CH, 256]
        utp = np.ascontiguousarray(
            usl.transpose(0, 2, 1).reshape(4, 128, M)
        ).astype(FP8)                                     # [4, 128, M]
        # masks: mask[d, l] = 1 iff this core c is in the level-l sibling
        # block of destination core d.
        mA = np.zeros((128, 8, B), dtype=BF16)
        mB = np.zeros((64, 8, B), dtype=BF16)
        for d in range(8):
            if (c // 4) == ((d // 4) ^ 1):
                mA[0:64, d, :] = 1.0   # level 0
            if (c // 2) == ((d // 2) ^ 1):
                mA[64:128, d, :] = 1.0  # level 1
            if c == d ^ 1:
                mB[:, d, :] = 1.0       # level 2
        in_maps.append({"xt": xt, "u": up, "ut": utp, "maskA": mA, "maskB": mB})
    return in_maps


last_results = None


def kernel(x, diag, u):
    global last_results
    from concourse.bass_utils import run_bass_kernel_spmd

    if "nc" not in _cached:
        _cached["nc"] = _build_bass()
    nc = _cached["nc"]

    in_maps = _pack_inputs(x, diag, u)
    res = run_bass_kernel_spmd(nc, in_maps, core_ids=list(range(NCORES)))
    last_results = res

    x2 = np.asarray(x, dtype=np.float32).reshape(B, N)
    d2 = np.asarray(diag, dtype=np.float32).reshape(1, N)
    y = d2 * x2
    inv = 1.0 / (USCALE * USCALE)
    for c in range(NCORES):
        y[:, c * M : (c + 1) * M] += res.results[c]["corr"].astype(np.float32) * inv
    return y.reshape(B, N, 1).astype(np.float32)


# revision 26
# speedup vs baseline: 1.2550x; 1.0287x over previous
"""HODLR matvec kernel for 8 TRN2 NeuronCores (Bass/Tile).

Sharding: node axis split into 8 contiguous slices of 32768 nodes.
Per core:
  projection  t[l,r,b] = sum_c u[l,c,r] * x[b,c]   (per block, all 8 levels)
              done in two passes: level-pairs (0,1) first, then (2,3),
              so the cross-core collective overlaps the second pass
  tree        combine L7-block partials up to coarser blocks
  A2A         exchange levels 0-2 sibling coefficients across cores
              (sender-side 0/1 masks make the combination core-invariant)
  expansion   corr[b,n] = sum_{l,r} u[l,n,r] * t_sib[l,r,b]
u/x are fed as fp8e4m3 (u scaled by USCALE; host divides the returned
correction by USCALE^2). The expansion runs fp8 DoubleRow matmuls that
contract two level-pairs (K=256) per instruction. Host computes diag*x
in fp32 and adds the device-computed correction.
"""

import os
import sys

sys.path.insert(0, "/opt/trn_rl_repo")

import numpy as np
import ml_dtypes

BF16 = ml_dtypes.bfloat16
FP8 = ml_dtypes.float8_e4m3

B = 64
N = 262144
NCORES = 8
M = N // NCORES          # 32768 nodes per core
R = 64
DEPTH = 8
CH = M // 128            # 256 chunks of 128 nodes
NB7 = M // 1024          # 32 L7 blocks (1024 nodes each)
USCALE = 64.0            # u is fed as u*USCALE in fp8 (e4m3 max finite 240)

_cached = {}


def _build_bass():
    import concourse.bacc as bacc
    import concourse.tile as tile
    import concourse.mybir as mybir
    from contextlib import ExitStack

    BF = mybir.dt.bfloat16
    F8 = mybir.dt.float8e4
    F32 = mybir.dt.float32
    ADD = mybir.AluOpType.add
    MULT = mybir.AluOpType.mult

    nc = bacc.Bacc(
        "TRN2",
        target_bir_lowering=False,
        debug=False,
        enable_asserts=False,
        num_devices=NCORES,
    )

    xt_d = nc.dram_tensor("xt", [128, CH, B], F8, kind="ExternalInput").ap()
    # u packed phase-major: [:, ph, k, :] holds level-pairs (2ph, 2ph+1)
    u_d = nc.dram_tensor("u", [128, 2, CH, 256], F8, kind="ExternalInput").ap()
    ut_d = nc.dram_tensor("ut", [4, 128, M], F8, kind="ExternalInput").ap()
    mA_d = nc.dram_tensor("maskA", [128, 8, B], BF, kind="ExternalInput").ap()
    mB_d = nc.dram_tensor("maskB", [64, 8, B], BF, kind="ExternalInput").ap()
    corr_d = nc.dram_tensor("corr", [B, M], BF, kind="ExternalOutput").ap()

    with tile.TileContext(nc) as tc, ExitStack() as ctx:
        const = ctx.enter_context(tc.tile_pool(name="const", bufs=1))
        upool = ctx.enter_context(tc.tile_pool(name="upool", bufs=5))
        pp = ctx.enter_context(tc.tile_pool(name="pp", bufs=4, space="PSUM"))
        treep = ctx.enter_context(tc.tile_pool(name="treep", bufs=1))
        statp = ctx.enter_context(tc.tile_pool(name="statp", bufs=1))
        utp = ctx.enter_context(tc.tile_pool(name="utp", bufs=3))
        ep = ctx.enter_context(tc.tile_pool(name="ep", bufs=4, space="PSUM"))
        yp = ctx.enter_context(tc.tile_pool(name="yp", bufs=3))
        dram = ctx.enter_context(tc.tile_pool(name="dram", bufs=1, space="DRAM"))

        xt = const.tile([128, CH, B], F8, tag="xt")
        for xq in range(4):
            nc.gpsimd.dma_start(
                xt[:, 64 * xq : 64 * (xq + 1), :],
                xt_d[:, 64 * xq : 64 * (xq + 1), :],
            )
        mA = const.tile([128, 8, B], BF, tag="mA")
        nc.scalar.dma_start(mA[:], mA_d[:])
        mB = const.tile([64, 8, B], BF, tag="mB")
        nc.scalar.dma_start(mB[:], mB_d[:])

        # ---------------- projection + per-pair trees ----------------
        # G[(q, sz)][j]: [128, 64] bf16; rows 0:64 -> level 2q, rows
        # 64:128 -> level 2q+1 of the j-th block of `sz` nodes (t^T
        # layout [(l,r), b], scaled by USCALE).
        need_top = {3: 2048, 2: 8192, 1: 32768, 0: 32768}
        G = {}

        def tcopy(i, out, in_):
            (nc.vector if i % 2 == 0 else nc.any).tensor_copy(out, in_)

        def tadd(i, out, a, b):
            if i % 2 == 0:
                nc.vector.tensor_tensor(out, a, b, op=ADD)
            else:
                nc.any.tensor_add(out, a, b)

        for ph, qs in ((0, (0, 1)), (1, (2, 3))):
            psums = {}  # (q, j) -> psum tile kept for direct tree combine
            for j in range(NB7):
                if j % 4 == 0:
                    u_t2 = upool.tile(
                        [128, 32, 256], F8, tag="u_in", name=f"u_t{ph}_{j}"
                    )
                    nc.sync.dma_start(
                        u_t2[:], u_d[:, ph, 8 * j : 8 * j + 32, :]
                    )
                u_t = u_t2[:, 8 * (j % 4) : 8 * (j % 4) + 8, :]
                for qi, q in enumerate(qs):
                    ps = pp.tile([128, B], F32, tag="proj", name=f"ps{q}_{j}")
                    for ki in range(8):
                        k = 8 * j + ki
                        nc.tensor.matmul(
                            ps[:],
                            u_t[:, ki, 128 * qi : 128 * qi + 128],
                            xt[:, k, :],
                            start=(ki == 0),
                            stop=(ki == 7),
                        )
                    if q == 3:
                        # pair 3 needs the 1024-blocks themselves
                        g = treep.tile([128, B], BF, tag=f"G3_1024_{j}")
                        tcopy(j, g[:], ps[:])
                        G.setdefault((3, 1024), []).append(g)
                    else:
                        if j % 2 == 0:
                            # drain even block to SBUF (an op may read at
                            # most one PSUM input)
                            tmp = treep.tile(
                                [128, B], BF, tag=f"T{q}_{j // 2}",
                                name=f"T{q}_{j // 2}",
                            )
                            tcopy(j // 2 + q, tmp[:], ps[:])
                            psums[(q, j)] = tmp
                        else:
                            g2 = treep.tile(
                                [128, B], BF, tag=f"G{q}_2048_{j // 2}"
                            )
                            tadd(
                                j // 2,
                                g2[:],
                                psums.pop((q, j - 1))[:],
                                ps[:],
                            )
                            G.setdefault((q, 2048), []).append(g2)
            # tree up
            for q in qs:
                if q == 3:
                    lst = G[(3, 1024)]
                    nxt = []
                    for j in range(len(lst) // 2):
                        g2 = treep.tile([128, B], BF, tag=f"G3_2048_{j}")
                        tadd(j, g2[:], lst[2 * j][:], lst[2 * j + 1][:])
                        nxt.append(g2)
                    G[(3, 2048)] = nxt
                sz = 2048
                while sz < need_top[q]:
                    lst = G[(q, sz)]
                    nxt = []
                    for j in range(len(lst) // 2):
                        g2 = treep.tile([128, B], BF, tag=f"G{q}_{2 * sz}_{j}")
                        tadd(j + q, g2[:], lst[2 * j][:], lst[2 * j + 1][:])
                        nxt.append(g2)
                    G[(q, 2 * sz)] = nxt
                    sz *= 2

            if ph == 0:
                # ------------- collective (overlaps phase 1) -------------
                # AllGather the raw level-0..2 partials; apply the
                # sibling-selection masks on the receive side (the
                # sibling relation is symmetric, so the same masks work).
                A = G[(0, 32768)][0]
                Bt = G[(1, 32768)][0]
                b_in = dram.tile([192, B], BF, tag="b_in")
                b_out = dram.tile(
                    [8, 192, B], BF, tag="b_out", addr_space="Shared"
                )
                nc.scalar.dma_start(b_in[0:128, :], A[:])
                nc.scalar.dma_start(b_in[128:192, :], Bt[0:64, :])
                nc.gpsimd.collective_compute(
                    "AllGather",
                    mybir.AluOpType.bypass,
                    replica_groups=[list(range(NCORES))],
                    ins=[b_in.opt()],
                    outs=[b_out.opt()],
                )
                recvA = statp.tile([128, 8, B], BF, tag="recvA")
                recvB = statp.tile([64, 8, B], BF, tag="recvB")
                for k in range(8):
                    nc.scalar.dma_start(recvA[:, k, :], b_out[k, 0:128, :])
                    nc.scalar.dma_start(recvB[:, k, :], b_out[k, 128:192, :])
                # masked receive-combine
                mskA = statp.tile([128, 8, B], BF, tag="mskA")
                mskB = statp.tile([64, 8, B], BF, tag="mskB")
                for k in range(8):
                    nc.vector.tensor_tensor(
                        mskA[:, k, :], recvA[:, k, :], mA[:, k, :], op=MULT
                    )
                    nc.vector.tensor_tensor(
                        mskB[:, k, :], recvB[:, k, :], mB[:, k, :], op=MULT
                    )
                tallA = statp.tile([128, B], BF, tag="tallA")
                tallB = statp.tile([64, B], BF, tag="tallB")
                nc.vector.tensor_tensor(
                    tallA[:], mskA[:, 0, :], mskA[:, 1, :], op=ADD
                )
                nc.vector.tensor_tensor(
                    tallB[:], mskB[:, 0, :], mskB[:, 1, :], op=ADD
                )
                for k in range(2, 8):
                    nc.vector.tensor_tensor(
                        tallA[:], tallA[:], mskA[:, k, :], op=ADD
                    )
                    nc.vector.tensor_tensor(
                        tallB[:], tallB[:], mskB[:, k, :], op=ADD
                    )

        # prefetch first expansion ut tiles early (scalar ring)
        ut_pre = []
        for f in range(2):
            t_ = utp.tile([128, 2, 4096], F8, tag=f"utf{f}", name=f"ut_pre{f}")
            for j in range(2):
                nc.scalar.dma_start(t_[:, j, :], ut_d[2 * f + j, :, 0:4096])
            ut_pre.append(t_)

        # ---------------- expansion stationaries (fp8, DoubleRow) --------
        # statf[q-pair-fuse] tiles [128, 2, B]: [:, j, :] holds the
        # stationary of level-pair (2*fuse + j); rows 0:64 = t_sib at the
        # even level of that pair, rows 64:128 = at the odd level.
        statf01 = []
        for m3 in range(2):
            s = statp.tile([128, 2, B], F8, tag=f"sf01_{m3}", name=f"sf01_{m3}")
            nc.vector.tensor_copy(s[:, 0, :], tallA[:])
            nc.vector.tensor_copy(s[0:64, 1, :], tallB[:])
            nc.vector.tensor_copy(
                s[64:128, 1, :], G[(1, 16384)][m3 ^ 1][64:128, :]
            )
            statf01.append(s)
        statf23 = []
        for m7 in range(NB7):
            s = statp.tile([128, 2, B], F8, tag=f"sf23_{m7}", name=f"sf23_{m7}")
            m5 = m7 // 4
            nc.vector.tensor_copy(
                s[0:64, 0, :], G[(2, 8192)][(m5 // 2) ^ 1][0:64, :]
            )
            nc.vector.tensor_copy(
                s[64:128, 0, :], G[(2, 4096)][m5 ^ 1][64:128, :]
            )
            nc.vector.tensor_copy(
                s[0:64, 1, :], G[(3, 2048)][(m7 // 2) ^ 1][0:64, :]
            )
            nc.vector.tensor_copy(
                s[64:128, 1, :], G[(3, 1024)][m7 ^ 1][64:128, :]
            )
            statf23.append(s)

        # ---------------- expansion (DoubleRow fp8) ----------------
        DR = mybir.MatmulPerfMode.DoubleRow
        for gb in range(8):  # 8 blocks of 8 groups x 512 nodes
            if gb == 0:
                ut_t = ut_pre
            else:
                ut_t = [
                    utp.tile(
                        [128, 2, 4096], F8, tag=f"utf{f}", name=f"utf{f}_{gb}"
                    )
                    for f in range(2)
                ]
                for f in range(2):
                    for j in range(2):
                        nc.scalar.dma_start(
                            ut_t[f][:, j, :],
                            ut_d[2 * f + j, :, 4096 * gb : 4096 * (gb + 1)],
                        )
            y_sb = yp.tile([B, 4096], BF, tag="y")
            for gg in range(8):
                g = 8 * gb + gg
                eps = ep.tile([B, 512], F32, tag="exp", name=f"eps{g}")
                sl = slice(512 * gg, 512 * (gg + 1))
                nc.tensor.matmul(
                    eps[:], statf23[g // 2][:], ut_t[1][:, :, sl],
                    start=True, stop=False, perf_mode=DR,
                )
                nc.tensor.matmul(
                    eps[:], statf01[g // 32][:], ut_t[0][:, :, sl],
                    start=False, stop=True, perf_mode=DR,
                )
                nc.vector.tensor_copy(y_sb[:, sl], eps[:])
            nc.sync.dma_start(corr_d[:, 4096 * gb : 4096 * (gb + 1)], y_sb[:])

    nc.compile()
    return nc


def _pack_inputs(x, diag, u):
    """Build per-core input maps. x (B,N,1) f32, u (DEPTH,N,R) f32."""
    in_maps = []
    x2 = np.asarray(x).reshape(B, N)
    u3 = np.asarray(u)
    for c in range(NCORES):
        base = c * M
        xsl = x2[:, base : base + M]                      # (B, M)
        usl = u3[:, base : base + M, :] * USCALE          # (8, M, 64)
        xt = np.ascontiguousarray(
            xsl.T.reshape(CH, 128, B).transpose(1, 0, 2)
        ).astype(FP8)                                     # [128, CH, B]
        up = np.ascontiguousarray(
            usl.transpose(1, 0, 2).reshape(M, 512)        # [n, l*64+r]
            .reshape(CH, 128, 2, 256)
            .transpose(1, 2, 0, 3)
        ).astype(FP8)                                     # [128, 2, CH, 256]
        utp = np.ascontiguousarray(
            usl.transpose(0, 2, 1).reshape(4, 128, M)
        ).astype(FP8)                                     # [4, 128, M]
        # masks: mask[d, l] = 1 iff this core c is in the level-l sibling
        # block of destination core d.
        mA = np.zeros((128, 8, B), dtype=BF16)
        mB = np.zeros((64, 8, B), dtype=BF16)
        for d in range(8):
            if (c // 4) == ((d // 4) ^ 1):
                mA[0:64, d, :] = 1.0   # level 0
            if (c // 2) == ((d // 2) ^ 1):
                mA[64:128, d, :] = 1.0  # level 1
            if c == d ^ 1:
                mB[:, d, :] = 1.0       # level 2
        in_maps.append({"xt": xt, "u": up, "ut": utp, "maskA": mA, "maskB": mB})
    return in_maps


last_results = None


def kernel(x, diag, u):
    global last_results
    from concourse.bass_utils import run_bass_kernel_spmd

    if "nc" not in _cached:
        _cached["nc"] = _build_bass()
    nc = _cached["nc"]

    in_maps = _pack_inputs(x, diag, u)
    res = run_bass_kernel_spmd(nc, in_maps, core_ids=list(range(NCORES)))
    last_results = res

    x2 = np.asarray(x, dtype=np.float32).reshape(B, N)
    d2 = np.asarray(diag, dtype=np.float32).reshape(1, N)
    y = d2 * x2
    inv = 1.0 / (USCALE * USCALE)
    for c in range(NCORES):
        y[:, c * M : (c + 1) * M] += res.results[c]["corr"].astype(np.float32) * inv
    return y.reshape(B, N, 1).astype(np.float32)
